# revision 1
# baseline (speedup 1.0000x reference)
"""KNN classifier layer (B=1024, N=32768, D=64, k=8, C=6) on 8 trn2 cores.

Strategy: shard queries (batch) across the 8 cores, 128 queries per core;
replicate the training set. Per core:
  key[q, n] = x_q . X_n - |X_n|^2/2   (monotone decreasing in distance^2)
computed as one augmented matmul ([x, 1] . [X, -|X|^2/2]), evacuated
PSUM->SBUF by the scalar engine. Top-8 per query = max8 over per-2048-chunk
top-8 candidates (union of chunk top-8s contains the global top-8). The
label histogram needs no indices: X_train is pre-sorted by class on the
host so each class is a contiguous column block; count of keys >= t_q
(t_q = 8th largest key) inside each block = number of top-8 neighbors of
that class. Fused is_ge+accumulate tensor_scalar does each block in one
DVE instruction.
"""

import numpy as np

B, N, D, K, C = 1024, 32768, 64, 8, 6
NCORES = 8
Q = B // NCORES  # queries per core

CHUNK = 512  # matmul moving free dim
MACRO = 2048  # max8 scan chunk
NEG = -1.0e30

_compiled = None


def _plan_layout(y_train: np.ndarray):
    """Class-sort permutation and even-width class blocks, padded to a
    multiple of MACRO columns."""
    perm = np.argsort(y_train, kind="stable")
    counts = np.bincount(y_train, minlength=C)
    widths = [int(c + (c & 1)) for c in counts]  # even block widths
    starts = np.concatenate([[0], np.cumsum(widths)]).astype(int)
    total = int(starts[-1])
    np_cols = ((total + MACRO - 1) // MACRO) * MACRO
    if np_cols < total + 0:
        np_cols += MACRO
    return perm, counts, widths, starts, np_cols


def _build_nc(np_cols: int, block_bounds, finalize: bool = True):
    import concourse.bacc as bacc
    import concourse.mybir as mybir
    from concourse.tile import TileContext

    f32 = mybir.dt.float32
    nc = bacc.Bacc(None, target_bir_lowering=False, debug=False)

    lhsT_d = nc.declare_dram_parameter("lhsT", [D + 1, Q], f32, isOutput=False)
    xm_d = nc.declare_dram_parameter("xm", [D + 1, np_cols], f32, isOutput=False)
    out_d = nc.declare_dram_parameter("out", [Q, C], f32, isOutput=True)

    n_chunks = np_cols // CHUNK
    n_macro = np_cols // MACRO
    per_macro = MACRO // CHUNK

    with TileContext(nc) as tc:
        with (
            tc.tile_pool(name="const", bufs=1) as const_pool,
            tc.tile_pool(name="rhs", bufs=4) as rhs_pool,
            tc.tile_pool(name="psum", bufs=2, space="PSUM") as psum_pool,
            tc.tile_pool(name="keys", bufs=1) as keys_pool,
            tc.tile_pool(name="small", bufs=1) as small_pool,
            tc.tile_pool(name="scr", bufs=2) as scr_pool,
        ):
            lhsT_sb = const_pool.tile([D + 1, Q], f32)
            nc.sync.dma_start(out=lhsT_sb, in_=lhsT_d[:, :])

            keys = keys_pool.tile([Q, np_cols], f32)
            cand = small_pool.tile([Q, n_macro * 8], f32)

            for m in range(n_macro):
                ps = psum_pool.tile([Q, MACRO], f32)
                for j in range(per_macro):
                    c = m * per_macro + j
                    rhs = rhs_pool.tile([D + 1, CHUNK], f32)
                    nc.sync.dma_start(
                        out=rhs, in_=xm_d[:, c * CHUNK : (c + 1) * CHUNK]
                    )
                    nc.tensor.matmul(
                        ps[:, j * CHUNK : (j + 1) * CHUNK],
                        lhsT=lhsT_sb,
                        rhs=rhs,
                        start=True,
                        stop=True,
                    )
                # evacuate PSUM -> SBUF on the scalar engine
                nc.scalar.copy(keys[:, m * MACRO : (m + 1) * MACRO], ps)
                # chunk top-8 candidates
                nc.vector.max(
                    out=cand[:, m * 8 : (m + 1) * 8],
                    in_=keys[:, m * MACRO : (m + 1) * MACRO],
                )

            v8 = small_pool.tile([Q, 8], f32)
            nc.vector.max(out=v8, in_=cand)
            tq = v8[:, 7:8]

            cnt = small_pool.tile([Q, C], f32)
            for ci, (s, e) in enumerate(block_bounds):
                scratch = scr_pool.tile([Q, max(w for _, w in
                                                [(b[0], b[1] - b[0]) for b in block_bounds])],
                                        f32, tag="scratch")
                nc.vector.tensor_scalar(
                    out=scratch[:, : e - s],
                    in0=keys[:, s:e],
                    scalar1=tq,
                    scalar2=None,
                    op0=mybir.AluOpType.is_ge,
                    op1=mybir.AluOpType.add,
                    accum_out=cnt[:, ci : ci + 1],
                )

            tot = small_pool.tile([Q, 1], f32)
            nc.vector.reduce_sum(tot, cnt, axis=mybir.AxisListType.X)
            rec = small_pool.tile([Q, 1], f32)
            nc.vector.reciprocal(rec, tot)
            prob = small_pool.tile([Q, C], f32)
            nc.vector.tensor_scalar(
                out=prob,
                in0=cnt,
                scalar1=rec,
                scalar2=None,
                op0=mybir.AluOpType.mult,
            )
            nc.sync.dma_start(out=out_d[:, :], in_=prob)

    if finalize:
        nc.finalize()
    return nc


def _prepare(x: np.ndarray, X_train: np.ndarray, y_train: np.ndarray):
    perm, counts, widths, starts, np_cols = _plan_layout(y_train)
    Xs = X_train[perm]  # [N, D] class-sorted
    t_sq = np.sum(Xs.astype(np.float32) * Xs.astype(np.float32), axis=1)

    xm = np.full((D + 1, np_cols), 0.0, dtype=np.float32)
    xm[D, :] = NEG  # dummy columns never win
    col = np.zeros(np_cols, dtype=bool)
    # scatter class blocks
    pos = 0
    bounds = []
    for ci in range(C):
        s = int(starts[ci])
        cnt_c = int(counts[ci])
        sel = slice(pos, pos + cnt_c)  # rows of Xs for this class (sorted)
        xm[:D, s : s + cnt_c] = Xs[sel].T
        xm[D, s : s + cnt_c] = -0.5 * t_sq[sel]
        bounds.append((s, s + widths[ci]))
        pos += cnt_c
    return xm, bounds, np_cols


def kernel(x: np.ndarray, X_train: np.ndarray, y_train: np.ndarray) -> np.ndarray:
    global _compiled
    from concourse.bass_utils import run_bass_kernel_spmd

    xm, bounds, np_cols = _prepare(x, X_train, y_train)

    if _compiled is None:
        _compiled = _build_nc(np_cols, bounds)
    nc = _compiled

    in_maps = []
    for core in range(NCORES):
        xc = x[core * Q : (core + 1) * Q].astype(np.float32)  # [Q, D]
        lhsT = np.concatenate([xc.T, np.ones((1, Q), np.float32)], axis=0)
        in_maps.append({"lhsT": lhsT, "xm": xm})

    res = run_bass_kernel_spmd(nc, in_maps, core_ids=list(range(NCORES)))
    out = np.concatenate([res.results[i]["out"] for i in range(NCORES)], axis=0)
    return out.astype(np.float32)

